# revision 9
# baseline (speedup 1.0000x reference)
"""Trainium2 Bass kernel for DFFormerMoELoss (ListMLE + pairwise logistic + MoE).

Math (per batch row, N=2048):
  mp = y_pred * masks ; mt = y_true * masks
  ListMLE: with m = max(mp), E_k = exp(mp_k - m),
    S_i = sum_{k: mt_k <= mt_i} E_k   (sort-free suffix-logsumexp identity)
    ll = (sum(mp) - sum_i log S_i) / N - m
    ranking = -mean_rows(ll)
  Pairwise: pos[j,i] = (mt_i > mt_j)
    with E'_k = exp(mp_k - m + 8):
    softplus(mp_j - mp_i) = ln(E'_i + E'_j) + (m - 8 - mp_i), so with
    q[j,i] = (E'_i + E'_j - 1) * pos[j,i]  (ln(q+1)=0 where masked),
    num = sum_{i,j} ln(q[j,i] + 1) - sum_i colcnt_i*(mp_i - m + 8)
      with colcnt_i = sum_j pos[j,i]   (the reference's neg-branch term is
      annihilated by its final *pos factor)
    cnt = sum_i colcnt_i ; pairwise = mean_rows(num / (cnt + 1e-12))
  total = ranking + 0.3 * pairwise + 0.03 * load_bal_loss

Sharding: data-parallel, 2 rows per core across 8 cores; per-core output is
[2,3] = (ll, num, cnt) per row; host reduces the 16 row-scalars.
"""

from contextlib import ExitStack

import numpy as np

import concourse.bacc as bacc
import concourse.bass as bass
import concourse.tile as tile
from concourse import mybir
from concourse.bass_utils import run_bass_kernel_spmd

B, N = 16, 2048
NCORES = 8
R = B // NCORES          # rows per core
NCH = N // 128           # 16 partition-chunks per row
NBLK = N // 512          # 4 psum column blocks
F32 = mybir.dt.float32
BF16 = mybir.dt.bfloat16
AF = mybir.ActivationFunctionType
ALU = mybir.AluOpType


def _emit(tc: tile.TileContext, ctx: ExitStack, yp, yt, mk, st):
    nc = tc.nc
    consts = ctx.enter_context(tc.tile_pool(name="consts", bufs=1))
    rowp = ctx.enter_context(tc.tile_pool(name="rowp", bufs=1))
    row2p = ctx.enter_context(tc.tile_pool(name="row2p", bufs=2))
    reppA = ctx.enter_context(tc.tile_pool(name="reppA", bufs=1))
    repp = ctx.enter_context(tc.tile_pool(name="repp", bufs=2))
    partp = ctx.enter_context(tc.tile_pool(name="partp", bufs=2))
    workp = ctx.enter_context(tc.tile_pool(name="workp", bufs=3))
    scalp = ctx.enter_context(tc.tile_pool(name="scalp", bufs=4))
    psA = ctx.enter_context(tc.tile_pool(name="psA", bufs=2, space=bass.MemorySpace.PSUM))
    psS = ctx.enter_context(tc.tile_pool(name="psS", bufs=4, space=bass.MemorySpace.PSUM))
    psC = ctx.enter_context(tc.tile_pool(name="psC", bufs=1, space=bass.MemorySpace.PSUM))

    ones_1x128 = consts.tile([1, 128], F32, tag="ones_1x128")
    nc.vector.memset(ones_1x128[:], 1.0)
    ones_col = consts.tile([128, 1], F32, tag="ones_col")
    nc.vector.memset(ones_col[:], 1.0)

    # ---------------- Phase A: load, mask, broadcast, exp (both rows) -------
    T_rep, P_rep, E_rep, T_part, P_part = [], [], [], [], []
    E_partf, EO, E_f, m_col, sum_p, mp_fs = [], [], [], [], [], []
    for r in range(R):
        pf = row2p.tile([1, N], F32, tag="raw")
        tf = row2p.tile([1, N], F32, tag="raw", name="tf")
        mf = rowp.tile([1, N], F32, tag="mf")
        nc.sync.dma_start(out=pf[:], in_=yp[r : r + 1, :])
        nc.sync.dma_start(out=tf[:], in_=yt[r : r + 1, :])
        nc.sync.dma_start(out=mf[:], in_=mk[r : r + 1, :])
        mp_f = row2p.tile([1, N], F32, tag="mp_f")
        mt_f = rowp.tile([1, N], F32, tag="mt_f")
        nc.vector.tensor_mul(mp_f[:], pf[:], mf[:])
        nc.vector.tensor_mul(mt_f[:], tf[:], mf[:])

        # partition-major layouts: element (c*128+q) -> [q, c]
        pq = partp.tile([128, NCH], F32, tag="pq")
        tq = partp.tile([128, NCH], F32, tag="tq")
        mq = partp.tile([128, NCH], F32, tag="mq")
        nc.sync.dma_start(out=pq[:], in_=yp[r].rearrange("(c p) -> p c", p=128))
        nc.sync.dma_start(out=tq[:], in_=yt[r].rearrange("(c p) -> p c", p=128))
        nc.sync.dma_start(out=mq[:], in_=mk[r].rearrange("(c p) -> p c", p=128))
        Pp = partp.tile([128, NCH], F32, tag="P_part")
        Tp = partp.tile([128, NCH], F32, tag="T_part")
        nc.vector.tensor_mul(Pp[:], pq[:], mq[:])
        nc.vector.tensor_mul(Tp[:], tq[:], mq[:])

        # replicate full row across 128 partitions via K=1 matmul
        Tr = repp.tile([128, N], F32, tag="T_rep")
        Pr = reppA.tile([128, N], F32, tag="P_rep")
        for src, dst in ((mt_f, Tr), (mp_f, Pr)):
            for b in range(NBLK):
                ps = psA.tile([128, 512], F32, tag="bcast", name=f"bc{r}{b}")
                nc.tensor.matmul(
                    ps[:], ones_1x128[:], src[0:1, b * 512 : (b + 1) * 512],
                    start=True, stop=True,
                )
                nc.vector.tensor_copy(dst[:, b * 512 : (b + 1) * 512], ps[:])

        mc = partp.tile([128, 1], F32, tag="m_col")
        nc.vector.reduce_max(mc[:], Pr[:], axis=mybir.AxisListType.X)
        mneg = partp.tile([128, 1], F32, tag="mneg")
        nc.vector.tensor_scalar_mul(mneg[:], mc[:], -1.0)
        sp = scalp.tile([1, 1], F32, tag="sum_p")
        nc.vector.reduce_sum(sp[:], mp_f[:], axis=mybir.AxisListType.X)

        mneg8 = partp.tile([128, 1], F32, tag="mneg8")
        nc.vector.tensor_scalar_add(mneg8[:], mneg[:], 8.0)

        Er = reppA.tile([128, N], F32, tag="E_rep")  # shifted: exp(p - m + 8)
        nc.scalar.activation(Er[:], Pr[:], AF.Exp, bias=mneg8[:], scale=1.0)
        Em1 = repp.tile([128, N], BF16, tag="EM1")   # E' - 1
        nc.vector.tensor_scalar_add(Em1[:], Er[:], -1.0)
        E8pf = partp.tile([128, NCH], F32, tag="E8_partf")
        nc.scalar.activation(E8pf[:], Pp[:], AF.Exp, bias=mneg8[:], scale=1.0)

        Epf = partp.tile([128, NCH], F32, tag="E_partf")  # unshifted (ListMLE)
        nc.scalar.activation(Epf[:], Pp[:], AF.Exp, bias=mneg[:], scale=1.0)
        Ef = row2p.tile([1, N], F32, tag="E_f")
        nc.scalar.activation(Ef[:], mp_f[:], AF.Exp, bias=mneg[0:1, :], scale=1.0)
        # interleaved bf16 lhsT: col 2c = E, col 2c+1 = 1  (contiguous slices)
        Eo = partp.tile([128, 2 * NCH], BF16, tag="EO")
        nc.vector.tensor_copy(Eo[:, 0 : 2 * NCH : 2], Epf[:])
        nc.vector.memset(Eo[:, 1 : 2 * NCH : 2], 1.0)

        T_rep.append(Tr); P_rep.append(Pr); E_rep.append(Em1)
        T_part.append(Tp); P_part.append(Pp); E_partf.append(E8pf); EO.append(Eo)
        E_f.append(Ef); m_col.append(mc); sum_p.append(sp); mp_fs.append(mp_f)

    # ------- Phase B (NxN sweep) + Phase D (stats), interleaved per row -----
    for r in range(R):
        np_ = partp.tile([128, NCH], F32, tag="num_part")
        sc_ps = [
            psS.tile([2, 512], F32, tag="sc", name=f"sc_r{r}b{b}")
            for b in range(NBLK)
        ]
        for c in range(NCH):
            pos = workp.tile([128, N], BF16, tag="pos")
            nc.vector.tensor_scalar(
                out=pos[:], in0=T_rep[r][:], scalar1=T_part[r][:, c : c + 1],
                scalar2=None, op0=ALU.is_gt,
            )
            q = workp.tile([128, N], BF16, tag="q")
            nc.vector.scalar_tensor_tensor(
                out=q[:], in0=E_rep[r][:], scalar=E_partf[r][:, c : c + 1],
                in1=pos[:], op0=ALU.add, op1=ALU.mult,
            )
            w = workp.tile([128, N], BF16, tag="w")
            nc.scalar.activation(
                w[:], q[:], AF.Ln, bias=1.0, scale=1.0,
                accum_out=np_[:, c : c + 1],
            )
            for b in range(NBLK):
                nc.tensor.matmul(
                    sc_ps[b][:], EO[r][:, 2 * c : 2 * c + 2],
                    pos[:, b * 512 : (b + 1) * 512],
                    start=(c == 0), stop=(c == NCH - 1),
                )

        # drain psum: row0 = SumE (add diagonal E_i), row1 = colcnt
        Ss = rowp.tile([1, N], F32, tag="SumE_sb")
        Cs = rowp.tile([1, N], F32, tag="colcnt_sb")
        for b in range(NBLK):
            sl = slice(b * 512, (b + 1) * 512)
            nc.vector.tensor_add(Ss[0:1, sl], sc_ps[b][0:1, :], E_f[r][0:1, sl])
            C2 = rowp.tile([2, 512], F32, tag="C2", name=f"C2_{r}{b}")
            nc.vector.tensor_copy(C2[:, :], sc_ps[b][:, :])
            nc.sync.dma_start(out=Cs[0:1, sl], in_=C2[1:2, :])

        # ---- stats: ln(S), corr, cnt, num, ll -> DMA out ----
        nc.scalar.activation(Ss[:], Ss[:], AF.Ln)  # logS in place
        slog = scalp.tile([1, 1], F32, tag="slog")
        nc.vector.reduce_sum(slog[:], Ss[:], axis=mybir.AxisListType.X)

        # corr = sum_i colcnt_i * (p_i - m + 8);  cnt = sum_i colcnt_i
        nc.vector.tensor_scalar(
            out=mp_fs[r][:], in0=mp_fs[r][:], scalar1=m_col[r][0:1, :],
            scalar2=8.0, op0=ALU.subtract, op1=ALU.add,
        )
        j3 = rowp.tile([1, N], F32, tag="j3")
        nc.vector.tensor_mul(j3[:], mp_fs[r][:], Cs[:])
        corr = scalp.tile([1, 1], F32, tag="corr")
        nc.vector.reduce_sum(corr[:], j3[:], axis=mybir.AxisListType.X)
        cnt = scalp.tile([1, 1], F32, tag="cnt")
        nc.vector.reduce_sum(cnt[:], Cs[:], axis=mybir.AxisListType.X)

        num_col = partp.tile([128, 1], F32, tag="num_col")
        nc.vector.reduce_sum(num_col[:], np_[:], axis=mybir.AxisListType.X)
        num_ps = psC.tile([1, 1], F32, tag="num_ps", name=f"num_ps{r}")
        nc.tensor.matmul(num_ps[:], num_col[:], ones_col[:], start=True, stop=True)
        num_fin = scalp.tile([1, 1], F32, tag="num_fin")
        nc.vector.tensor_sub(num_fin[:], num_ps[:], corr[:])

        d1 = scalp.tile([1, 1], F32, tag="d1")
        nc.vector.tensor_sub(d1[:], sum_p[r][:], slog[:])
        ll = scalp.tile([1, 1], F32, tag="ll")
        nc.vector.scalar_tensor_tensor(
            out=ll[:], in0=d1[:], scalar=1.0 / N, in1=m_col[r][0:1, :],
            op0=ALU.mult, op1=ALU.subtract,
        )
        stt = scalp.tile([1, 3], F32, tag="stt")
        nc.scalar.copy(stt[0:1, 0:1], ll[:])
        nc.scalar.copy(stt[0:1, 1:2], num_fin[:])
        nc.scalar.copy(stt[0:1, 2:3], cnt[:])
        nc.sync.dma_start(out=st[r : r + 1, :], in_=stt[:])


_CACHED = None


def _build():
    global _CACHED
    if _CACHED is not None:
        return _CACHED
    nc = bacc.Bacc(
        "TRN2", target_bir_lowering=False, debug=False,
        enable_asserts=False, num_devices=NCORES,
    )
    yp = nc.dram_tensor("yp", [R, N], F32, kind="ExternalInput").ap()
    yt = nc.dram_tensor("yt", [R, N], F32, kind="ExternalInput").ap()
    mk = nc.dram_tensor("mk", [R, N], F32, kind="ExternalInput").ap()
    st = nc.dram_tensor("stats", [R, 3], F32, kind="ExternalOutput").ap()
    with tile.TileContext(nc) as tc, ExitStack() as ctx:
        _emit(tc, ctx, yp, yt, mk, st)
    nc.compile()
    _CACHED = nc
    return nc


def make_in_maps(y_pred, y_true, masks):
    return [
        {
            "yp": np.ascontiguousarray(y_pred[i * R : (i + 1) * R], dtype=np.float32),
            "yt": np.ascontiguousarray(y_true[i * R : (i + 1) * R], dtype=np.float32),
            "mk": np.ascontiguousarray(masks[i * R : (i + 1) * R], dtype=np.float32),
        }
        for i in range(NCORES)
    ]


def combine(stats, load_bal_loss):
    """stats: [NCORES, R, 3] -> scalar loss (matches reference combine)."""
    stats = np.asarray(stats, dtype=np.float64)
    ll = stats[:, :, 0].ravel()
    num = stats[:, :, 1].ravel()
    cnt = stats[:, :, 2].ravel()
    ranking = -np.mean(ll)
    pairwise = np.mean(num / (cnt + 1e-12))
    total = ranking + 0.3 * pairwise + 0.03 * float(np.squeeze(load_bal_loss))
    return np.float32(total)


def run_on_hw(in_maps, trace=False):
    nc = _build()
    return run_bass_kernel_spmd(nc, in_maps, list(range(NCORES)), trace=trace)


def kernel(y_pred, y_true, masks, load_bal_loss):
    res = run_on_hw(make_in_maps(y_pred, y_true, masks))
    stats = np.stack([res.results[i]["stats"] for i in range(NCORES)])
    return combine(stats, load_bal_loss)


# revision 11
# speedup vs baseline: 1.6261x; 1.6261x over previous
"""Trainium2 Bass kernel for DFFormerMoELoss (ListMLE + pairwise logistic + MoE).

Math (per batch row, N=2048):
  mp = y_pred * masks ; mt = y_true * masks
  ListMLE: with m = max(mp), E_k = exp(mp_k - m),
    S_i = sum_{k: mt_k <= mt_i} E_k   (sort-free suffix-logsumexp identity)
    ll = (sum(mp) - sum_i log S_i) / N - m
    ranking = -mean_rows(ll)
  Pairwise: pos[j,i] = (mt_i > mt_j). With E'_k = exp(mp_k - m + 8):
    softplus(mp_j - mp_i) = ln(E'_i + E'_j) + (m - 8 - mp_i)
    w_pre[j,i] = ln(E'_i + E'_j) is symmetric and pos[j,i] + pos[i,j] = 1
    for i != j (no ties), so
      sum pos*w_pre = (S_all - S_diag)/2,
      S_all = sum_{i,j} w_pre,  S_diag = N*ln2 + sum(mp) - N*(m-8)
    num = (S_all - S_diag)/2 - sum_i colcnt_i*(mp_i - m + 8),
      colcnt_i = sum_j pos[j,i]  (the reference's neg-branch term is
      annihilated by its final *pos factor)
    cnt = sum_i colcnt_i ; pairwise = mean_rows(num / (cnt + 1e-12))
  total = ranking + 0.3 * pairwise + 0.03 * load_bal_loss

Sharding: data-parallel, 2 rows per core across 8 cores; per-core output is
[2,3] = (ll, num, cnt) per row; host reduces the 16 row-scalars.
"""

import math
from contextlib import ExitStack

import numpy as np

import concourse.bacc as bacc
import concourse.bass as bass
import concourse.tile as tile
from concourse import mybir
from concourse.bass_utils import run_bass_kernel_spmd

B, N = 16, 2048
NCORES = 8
R = B // NCORES          # rows per core
NCH = N // 128           # 16 partition-chunks per row
NBLK = N // 512          # 4 psum column blocks
SHIFT = 8.0
F32 = mybir.dt.float32
BF16 = mybir.dt.bfloat16
AF = mybir.ActivationFunctionType
ALU = mybir.AluOpType


def _emit(tc: tile.TileContext, ctx: ExitStack, yp, yt, mk, st):
    nc = tc.nc
    consts = ctx.enter_context(tc.tile_pool(name="consts", bufs=1))
    rowp = ctx.enter_context(tc.tile_pool(name="rowp", bufs=1))
    row2p = ctx.enter_context(tc.tile_pool(name="row2p", bufs=2))
    repp = ctx.enter_context(tc.tile_pool(name="repp", bufs=2))
    partp = ctx.enter_context(tc.tile_pool(name="partp", bufs=2))
    workp = ctx.enter_context(tc.tile_pool(name="workp", bufs=3))
    scalp = ctx.enter_context(tc.tile_pool(name="scalp", bufs=4))
    psA = ctx.enter_context(tc.tile_pool(name="psA", bufs=2, space=bass.MemorySpace.PSUM))
    psS = ctx.enter_context(tc.tile_pool(name="psS", bufs=4, space=bass.MemorySpace.PSUM))
    psC = ctx.enter_context(tc.tile_pool(name="psC", bufs=2, space=bass.MemorySpace.PSUM))

    ones_1x128 = consts.tile([1, 128], F32, tag="ones_1x128")
    nc.vector.memset(ones_1x128[:], 1.0)
    ones_1x128b = consts.tile([1, 128], BF16, tag="ones_1x128b")
    nc.vector.memset(ones_1x128b[:], 1.0)
    ones_col = consts.tile([128, 1], F32, tag="ones_col")
    nc.vector.memset(ones_col[:], 1.0)

    # ---------------- Phase A: load, mask, broadcast, exp (both rows) -------
    T_rep, E8_rep, T_part, P_part, E8_part = [], [], [], [], []
    EO, E_f, m_s, m8_s, sum_p, mp_fs = [], [], [], [], [], []
    for r in range(R):
        pf = row2p.tile([1, N], F32, tag="raw")
        tf = row2p.tile([1, N], F32, tag="raw", name="tf")
        mf = rowp.tile([1, N], F32, tag="mf")
        nc.sync.dma_start(out=pf[:], in_=yp[r : r + 1, :])
        nc.sync.dma_start(out=tf[:], in_=yt[r : r + 1, :])
        nc.sync.dma_start(out=mf[:], in_=mk[r : r + 1, :])
        mp_f = row2p.tile([1, N], F32, tag="mp_f")
        mt_f = rowp.tile([1, N], F32, tag="mt_f")
        nc.vector.tensor_mul(mp_f[:], pf[:], mf[:])
        nc.vector.tensor_mul(mt_f[:], tf[:], mf[:])

        # partition-major layouts: element (c*128+q) -> [q, c]
        pq = partp.tile([128, NCH], F32, tag="pq")
        tq = partp.tile([128, NCH], F32, tag="tq")
        mq = partp.tile([128, NCH], F32, tag="mq")
        nc.sync.dma_start(out=pq[:], in_=yp[r].rearrange("(c p) -> p c", p=128))
        nc.sync.dma_start(out=tq[:], in_=yt[r].rearrange("(c p) -> p c", p=128))
        nc.sync.dma_start(out=mq[:], in_=mk[r].rearrange("(c p) -> p c", p=128))
        Pp = partp.tile([128, NCH], F32, tag="P_part")
        Tp = partp.tile([128, NCH], F32, tag="T_part")
        nc.vector.tensor_mul(Pp[:], pq[:], mq[:])
        nc.vector.tensor_mul(Tp[:], tq[:], mq[:])

        # row max m and derived scalars
        m1 = scalp.tile([1, 1], F32, tag="m1")
        nc.vector.reduce_max(m1[:], mp_f[:], axis=mybir.AxisListType.X)
        m8 = scalp.tile([1, 1], F32, tag="m8")
        nc.vector.tensor_scalar_add(m8[:], m1[:], -SHIFT)       # m - 8
        mm2 = scalp.tile([1, 2], F32, tag="mm2")
        nc.vector.tensor_scalar_mul(mm2[0:1, 0:1], m1[:], -1.0)  # -m
        nc.vector.tensor_scalar_mul(mm2[0:1, 1:2], m8[:], -1.0)  # 8 - m
        mps = psC.tile([128, 2], F32, tag="tiny", name=f"mps{r}")
        nc.tensor.matmul(mps[:], ones_1x128[:], mm2[:], start=True, stop=True)
        mcols = partp.tile([128, 2], F32, tag="mcols")
        nc.vector.tensor_copy(mcols[:], mps[:])
        mneg, mneg8 = mcols[:, 0:1], mcols[:, 1:2]

        # sum(mp) via partition layout
        spc = partp.tile([128, 1], F32, tag="spc")
        nc.vector.reduce_sum(spc[:], Pp[:], axis=mybir.AxisListType.X)
        spp = psC.tile([1, 1], F32, tag="tiny", name=f"spp{r}")
        nc.tensor.matmul(spp[:], spc[:], ones_col[:], start=True, stop=True)
        sp = scalp.tile([1, 1], F32, tag="sum_p")
        nc.scalar.copy(sp[:], spp[:])

        # exps: unshifted for ListMLE, shifted (+8) for pairwise ln
        Epf = partp.tile([128, NCH], F32, tag="E_partf")
        nc.scalar.activation(Epf[:], Pp[:], AF.Exp, bias=mneg, scale=1.0)
        E8p = partp.tile([128, NCH], F32, tag="E8_part")
        nc.scalar.activation(E8p[:], Pp[:], AF.Exp, bias=mneg8, scale=1.0)
        Ef = row2p.tile([1, N], F32, tag="E_f")
        nc.scalar.activation(Ef[:], mp_f[:], AF.Exp, bias=mm2[0:1, 0:1], scale=1.0)
        E8f = rowp.tile([1, N], BF16, tag="E8_f")
        nc.scalar.activation(E8f[:], mp_f[:], AF.Exp, bias=mm2[0:1, 1:2], scale=1.0)
        # interleaved bf16 lhsT: col 2c = E, col 2c+1 = 1  (contiguous slices)
        Eo = partp.tile([128, 2 * NCH], BF16, tag="EO")
        nc.vector.tensor_copy(Eo[:, 0 : 2 * NCH : 2], Epf[:])
        nc.vector.memset(Eo[:, 1 : 2 * NCH : 2], 1.0)

        # replicate rows across 128 partitions via K=1 matmul
        Tr = repp.tile([128, N], F32, tag="T_rep")
        for b in range(NBLK):
            ps = psA.tile([128, 512], F32, tag="bcast", name=f"bcT{r}{b}")
            nc.tensor.matmul(
                ps[:], ones_1x128[:], mt_f[0:1, b * 512 : (b + 1) * 512],
                start=True, stop=True,
            )
            nc.vector.tensor_copy(Tr[:, b * 512 : (b + 1) * 512], ps[:])
        E8r = repp.tile([128, N], BF16, tag="E8_rep")
        for b in range(NBLK):
            ps = psA.tile([128, 512], F32, tag="bcast", name=f"bcE{r}{b}")
            nc.tensor.matmul(
                ps[:], ones_1x128b[:], E8f[0:1, b * 512 : (b + 1) * 512],
                start=True, stop=True,
            )
            nc.vector.tensor_copy(E8r[:, b * 512 : (b + 1) * 512], ps[:])

        T_rep.append(Tr); E8_rep.append(E8r); T_part.append(Tp); P_part.append(Pp)
        E8_part.append(E8p); EO.append(Eo); E_f.append(Ef)
        m_s.append(m1); m8_s.append(m8); sum_p.append(sp); mp_fs.append(mp_f)

    # ------- Phase B (NxN sweep) + Phase D (stats), interleaved per row -----
    for r in range(R):
        np_ = partp.tile([128, NCH], F32, tag="num_part")
        sc_ps = [
            psS.tile([2, 512], F32, tag="sc", name=f"sc_r{r}b{b}")
            for b in range(NBLK)
        ]
        for c in range(NCH):
            pos = workp.tile([128, N], BF16, tag="pos")
            nc.vector.tensor_scalar(
                out=pos[:], in0=T_rep[r][:], scalar1=T_part[r][:, c : c + 1],
                scalar2=None, op0=ALU.is_gt,
            )
            w = workp.tile([128, N], BF16, tag="w")
            nc.scalar.activation(
                w[:], E8_rep[r][:], AF.Ln, bias=E8_part[r][:, c : c + 1],
                scale=1.0, accum_out=np_[:, c : c + 1],
            )
            for b in range(NBLK):
                nc.tensor.matmul(
                    sc_ps[b][:], EO[r][:, 2 * c : 2 * c + 2],
                    pos[:, b * 512 : (b + 1) * 512],
                    start=(c == 0), stop=(c == NCH - 1),
                )

        # drain psum: row0 = SumE (add diagonal E_i), row1 = colcnt
        Ss = rowp.tile([1, N], F32, tag="SumE_sb")
        Cs = rowp.tile([1, N], F32, tag="colcnt_sb")
        for b in range(NBLK):
            sl = slice(b * 512, (b + 1) * 512)
            nc.vector.tensor_add(Ss[0:1, sl], sc_ps[b][0:1, :], E_f[r][0:1, sl])
            C2 = rowp.tile([2, 512], F32, tag="C2", name=f"C2_{r}{b}")
            nc.vector.tensor_copy(C2[:, :], sc_ps[b][:, :])
            nc.sync.dma_start(out=Cs[0:1, sl], in_=C2[1:2, :])

        # ---- stats ----
        # slog = sum_i ln(S_i)  (Ln in place, accum gives the sum)
        slog = scalp.tile([1, 1], F32, tag="slog")
        nc.scalar.activation(Ss[:], Ss[:], AF.Ln, accum_out=slog[:])

        # corr = sum_i colcnt_i * (mp_i - (m-8));  cnt = sum_i colcnt_i
        j3 = rowp.tile([1, N], F32, tag="j3")
        corr = scalp.tile([1, 1], F32, tag="corr")
        nc.vector.scalar_tensor_tensor(
            out=j3[:], in0=mp_fs[r][:], scalar=m8_s[r][:], in1=Cs[:],
            op0=ALU.subtract, op1=ALU.mult, accum_out=corr[:],
        )
        cnt = scalp.tile([1, 1], F32, tag="cnt")
        nc.vector.reduce_sum(cnt[:], Cs[:], axis=mybir.AxisListType.X)

        # S_all from the ACT accumulators
        sall_col = partp.tile([128, 1], F32, tag="sall_col")
        nc.vector.reduce_sum(sall_col[:], np_[:], axis=mybir.AxisListType.X)
        sall_ps = psC.tile([1, 1], F32, tag="tiny", name=f"sall_ps{r}")
        nc.tensor.matmul(sall_ps[:], sall_col[:], ones_col[:], start=True, stop=True)

        # S_diag = N*ln2 + sum_p - N*(m-8)
        sdiag = scalp.tile([1, 1], F32, tag="sdiag")
        nc.vector.scalar_tensor_tensor(
            out=sdiag[:], in0=m8_s[r][:], scalar=-float(N), in1=sum_p[r][:],
            op0=ALU.mult, op1=ALU.add,
        )
        nc.vector.tensor_scalar_add(sdiag[:], sdiag[:], float(N * math.log(2.0)))
        # num = (S_all - S_diag)/2 - corr
        haldif = scalp.tile([1, 1], F32, tag="haldif")
        nc.vector.tensor_sub(haldif[:], sall_ps[:], sdiag[:])
        num_fin = scalp.tile([1, 1], F32, tag="num_fin")
        nc.vector.scalar_tensor_tensor(
            out=num_fin[:], in0=haldif[:], scalar=0.5, in1=corr[:],
            op0=ALU.mult, op1=ALU.subtract,
        )

        # ll = (sum_p - slog)/N - m
        d1 = scalp.tile([1, 1], F32, tag="d1")
        nc.vector.tensor_sub(d1[:], sum_p[r][:], slog[:])
        ll = scalp.tile([1, 1], F32, tag="ll")
        nc.vector.scalar_tensor_tensor(
            out=ll[:], in0=d1[:], scalar=1.0 / N, in1=m_s[r][:],
            op0=ALU.mult, op1=ALU.subtract,
        )
        stt = scalp.tile([1, 3], F32, tag="stt")
        nc.scalar.copy(stt[0:1, 0:1], ll[:])
        nc.scalar.copy(stt[0:1, 1:2], num_fin[:])
        nc.scalar.copy(stt[0:1, 2:3], cnt[:])
        nc.sync.dma_start(out=st[r : r + 1, :], in_=stt[:])


_CACHED = None


def _build():
    global _CACHED
    if _CACHED is not None:
        return _CACHED
    nc = bacc.Bacc(
        "TRN2", target_bir_lowering=False, debug=False,
        enable_asserts=False, num_devices=NCORES,
    )
    yp = nc.dram_tensor("yp", [R, N], F32, kind="ExternalInput").ap()
    yt = nc.dram_tensor("yt", [R, N], F32, kind="ExternalInput").ap()
    mk = nc.dram_tensor("mk", [R, N], F32, kind="ExternalInput").ap()
    st = nc.dram_tensor("stats", [R, 3], F32, kind="ExternalOutput").ap()
    with tile.TileContext(nc) as tc, ExitStack() as ctx:
        _emit(tc, ctx, yp, yt, mk, st)
    nc.compile()
    _CACHED = nc
    return nc


def make_in_maps(y_pred, y_true, masks):
    return [
        {
            "yp": np.ascontiguousarray(y_pred[i * R : (i + 1) * R], dtype=np.float32),
            "yt": np.ascontiguousarray(y_true[i * R : (i + 1) * R], dtype=np.float32),
            "mk": np.ascontiguousarray(masks[i * R : (i + 1) * R], dtype=np.float32),
        }
        for i in range(NCORES)
    ]


def combine(stats, load_bal_loss):
    """stats: [NCORES, R, 3] -> scalar loss (matches reference combine)."""
    stats = np.asarray(stats, dtype=np.float64)
    ll = stats[:, :, 0].ravel()
    num = stats[:, :, 1].ravel()
    cnt = stats[:, :, 2].ravel()
    ranking = -np.mean(ll)
    pairwise = np.mean(num / (cnt + 1e-12))
    total = ranking + 0.3 * pairwise + 0.03 * float(np.squeeze(load_bal_loss))
    return np.float32(total)


def run_on_hw(in_maps, trace=False):
    nc = _build()
    return run_bass_kernel_spmd(nc, in_maps, list(range(NCORES)), trace=trace)


def kernel(y_pred, y_true, masks, load_bal_loss):
    res = run_on_hw(make_in_maps(y_pred, y_true, masks))
    stats = np.stack([res.results[i]["stats"] for i in range(NCORES)])
    return combine(stats, load_bal_loss)


# revision 12
# speedup vs baseline: 1.6649x; 1.0239x over previous
"""Trainium2 Bass kernel for DFFormerMoELoss (ListMLE + pairwise logistic + MoE).

Math (per batch row, N=2048):
  mp = y_pred * masks ; mt = y_true * masks
  ListMLE: with m = max(mp), E_k = exp(mp_k - m),
    S_i = sum_{k: mt_k <= mt_i} E_k   (sort-free suffix-logsumexp identity)
    ll = (sum(mp) - sum_i log S_i) / N - m
    ranking = -mean_rows(ll)
  Pairwise: pos[j,i] = (mt_i > mt_j). With E'_k = exp(mp_k - m + 8):
    softplus(mp_j - mp_i) = ln(E'_i + E'_j) + (m - 8 - mp_i)
    w_pre[j,i] = ln(E'_i + E'_j) is symmetric and pos[j,i] + pos[i,j] = 1
    for i != j (no ties), so
      sum pos*w_pre = (S_all - S_diag)/2,
      S_all = sum_{i,j} w_pre,  S_diag = N*ln2 + sum(mp) - N*(m-8)
    num = (S_all - S_diag)/2 - sum_i colcnt_i*(mp_i - m + 8),
      colcnt_i = sum_j pos[j,i]  (the reference's neg-branch term is
      annihilated by its final *pos factor)
    cnt = sum_i colcnt_i ; pairwise = mean_rows(num / (cnt + 1e-12))
  total = ranking + 0.3 * pairwise + 0.03 * load_bal_loss

Sharding: data-parallel, 2 rows per core across 8 cores; per-core output is
[2,3] = (ll, num, cnt) per row; host reduces the 16 row-scalars.
"""

import math
from contextlib import ExitStack

import numpy as np

import concourse.bacc as bacc
import concourse.bass as bass
import concourse.tile as tile
from concourse import mybir
from concourse.bass_utils import run_bass_kernel_spmd

B, N = 16, 2048
NCORES = 8
R = B // NCORES          # rows per core
NCH = N // 128           # 16 partition-chunks per row
NBLK = N // 512          # 4 psum column blocks
SHIFT = 8.0
F32 = mybir.dt.float32
BF16 = mybir.dt.bfloat16
AF = mybir.ActivationFunctionType
ALU = mybir.AluOpType


def _emit(tc: tile.TileContext, ctx: ExitStack, yp, yt, mk, st):
    nc = tc.nc
    consts = ctx.enter_context(tc.tile_pool(name="consts", bufs=1))
    rowp = ctx.enter_context(tc.tile_pool(name="rowp", bufs=1))
    row2p = ctx.enter_context(tc.tile_pool(name="row2p", bufs=2))
    repp = ctx.enter_context(tc.tile_pool(name="repp", bufs=2))
    partp = ctx.enter_context(tc.tile_pool(name="partp", bufs=2))
    workp = ctx.enter_context(tc.tile_pool(name="workp", bufs=3))
    scalp = ctx.enter_context(tc.tile_pool(name="scalp", bufs=4))
    psS = ctx.enter_context(tc.tile_pool(name="psS", bufs=6, space=bass.MemorySpace.PSUM))
    psC = ctx.enter_context(tc.tile_pool(name="psC", bufs=2, space=bass.MemorySpace.PSUM))

    ones_col = consts.tile([128, 1], F32, tag="ones_col")
    nc.vector.memset(ones_col[:], 1.0)

    # ---------------- Phase A: load, mask, broadcast, exp (both rows) -------
    T_rep, E8_rep, T_part, P_part, E8_part = [], [], [], [], []
    EO, E_f, m_s, m8_s, sum_p, mp_fs = [], [], [], [], [], []
    for r in range(R):
        pf = row2p.tile([1, N], F32, tag="raw")
        tf = row2p.tile([1, N], F32, tag="raw", name="tf")
        mf = rowp.tile([1, N], F32, tag="mf")
        nc.sync.dma_start(out=pf[:], in_=yp[r : r + 1, :])
        nc.sync.dma_start(out=tf[:], in_=yt[r : r + 1, :])
        nc.sync.dma_start(out=mf[:], in_=mk[r : r + 1, :])
        mp_f = row2p.tile([1, N], F32, tag="mp_f")
        mt_f = rowp.tile([1, N], F32, tag="mt_f")
        nc.vector.tensor_mul(mp_f[:], pf[:], mf[:])
        nc.vector.tensor_mul(mt_f[:], tf[:], mf[:])

        # partition-major layouts: element (c*128+q) -> [q, c]
        pq = partp.tile([128, NCH], F32, tag="pq")
        tq = partp.tile([128, NCH], F32, tag="tq")
        mq = partp.tile([128, NCH], F32, tag="mq")
        nc.sync.dma_start(out=pq[:], in_=yp[r].rearrange("(c p) -> p c", p=128))
        nc.sync.dma_start(out=tq[:], in_=yt[r].rearrange("(c p) -> p c", p=128))
        nc.sync.dma_start(out=mq[:], in_=mk[r].rearrange("(c p) -> p c", p=128))
        Pp = partp.tile([128, NCH], F32, tag="P_part")
        Tp = partp.tile([128, NCH], F32, tag="T_part")
        nc.vector.tensor_mul(Pp[:], pq[:], mq[:])
        nc.vector.tensor_mul(Tp[:], tq[:], mq[:])

        # row max m and derived scalars
        m1 = scalp.tile([1, 1], F32, tag="m1")
        nc.vector.reduce_max(m1[:], mp_f[:], axis=mybir.AxisListType.X)
        m8 = scalp.tile([1, 1], F32, tag="m8")
        nc.vector.tensor_scalar_add(m8[:], m1[:], -SHIFT)       # m - 8
        mm2 = scalp.tile([1, 2], F32, tag="mm2")
        nc.vector.tensor_scalar_mul(mm2[0:1, 0:1], m1[:], -1.0)  # -m
        nc.vector.tensor_scalar_mul(mm2[0:1, 1:2], m8[:], -1.0)  # 8 - m
        mcols = partp.tile([128, 2], F32, tag="mcols")
        nc.gpsimd.partition_broadcast(mcols[:], mm2[:])
        mneg, mneg8 = mcols[:, 0:1], mcols[:, 1:2]

        # sum(mp) via partition layout
        spc = partp.tile([128, 1], F32, tag="spc")
        nc.vector.reduce_sum(spc[:], Pp[:], axis=mybir.AxisListType.X)
        spp = psC.tile([1, 1], F32, tag="tiny", name=f"spp{r}")
        nc.tensor.matmul(spp[:], spc[:], ones_col[:], start=True, stop=True)
        sp = scalp.tile([1, 1], F32, tag="sum_p")
        nc.scalar.copy(sp[:], spp[:])

        # exps: unshifted for ListMLE, shifted (+8) for pairwise ln
        Epf = partp.tile([128, NCH], F32, tag="E_partf")
        nc.scalar.activation(Epf[:], Pp[:], AF.Exp, bias=mneg, scale=1.0)
        E8p = partp.tile([128, NCH], F32, tag="E8_part")
        nc.scalar.activation(E8p[:], Pp[:], AF.Exp, bias=mneg8, scale=1.0)
        Ef = row2p.tile([1, N], F32, tag="E_f")
        nc.scalar.activation(Ef[:], mp_f[:], AF.Exp, bias=mm2[0:1, 0:1], scale=1.0)
        E8f = rowp.tile([1, N], BF16, tag="E8_f")
        nc.scalar.activation(E8f[:], mp_f[:], AF.Exp, bias=mm2[0:1, 1:2], scale=1.0)
        # interleaved bf16 lhsT: col 2c = E, col 2c+1 = 1  (contiguous slices)
        Eo = partp.tile([128, 2 * NCH], BF16, tag="EO")
        nc.vector.tensor_copy(Eo[:, 0 : 2 * NCH : 2], Epf[:])
        nc.vector.memset(Eo[:, 1 : 2 * NCH : 2], 1.0)

        # replicate rows across 128 partitions on the (idle) Pool engine
        Tr = repp.tile([128, N], F32, tag="T_rep")
        nc.gpsimd.partition_broadcast(Tr[:], mt_f[:])
        E8r = repp.tile([128, N], BF16, tag="E8_rep")
        nc.gpsimd.partition_broadcast(E8r[:], E8f[:])

        T_rep.append(Tr); E8_rep.append(E8r); T_part.append(Tp); P_part.append(Pp)
        E8_part.append(E8p); EO.append(Eo); E_f.append(Ef)
        m_s.append(m1); m8_s.append(m8); sum_p.append(sp); mp_fs.append(mp_f)

    # ------- Phase B (NxN sweep) + Phase D (stats), interleaved per row -----
    for r in range(R):
        np_ = partp.tile([128, NCH], F32, tag="num_part")
        sc_ps = [
            psS.tile([2, 512], F32, tag="sc", name=f"sc_r{r}b{b}")
            for b in range(NBLK)
        ]
        for c in range(NCH):
            pos = workp.tile([128, N], BF16, tag="pos")
            nc.vector.tensor_scalar(
                out=pos[:], in0=T_rep[r][:], scalar1=T_part[r][:, c : c + 1],
                scalar2=None, op0=ALU.is_gt,
            )
            w = workp.tile([128, N], BF16, tag="w")
            nc.scalar.activation(
                w[:], E8_rep[r][:], AF.Ln, bias=E8_part[r][:, c : c + 1],
                scale=1.0, accum_out=np_[:, c : c + 1],
            )
            for b in range(NBLK):
                nc.tensor.matmul(
                    sc_ps[b][:], EO[r][:, 2 * c : 2 * c + 2],
                    pos[:, b * 512 : (b + 1) * 512],
                    start=(c == 0), stop=(c == NCH - 1),
                )

        # drain psum: row0 = SumE (add diagonal E_i), row1 = colcnt
        Ss = rowp.tile([1, N], F32, tag="SumE_sb")
        Cs = rowp.tile([1, N], F32, tag="colcnt_sb")
        for b in range(NBLK):
            sl = slice(b * 512, (b + 1) * 512)
            nc.vector.tensor_add(Ss[0:1, sl], sc_ps[b][0:1, :], E_f[r][0:1, sl])
            C2 = rowp.tile([2, 512], F32, tag="C2", name=f"C2_{r}{b}")
            nc.vector.tensor_copy(C2[:, :], sc_ps[b][:, :])
            nc.sync.dma_start(out=Cs[0:1, sl], in_=C2[1:2, :])

        # ---- stats ----
        # slog = sum_i ln(S_i)  (Ln in place, accum gives the sum)
        slog = scalp.tile([1, 1], F32, tag="slog")
        nc.scalar.activation(Ss[:], Ss[:], AF.Ln, accum_out=slog[:])

        # corr = sum_i colcnt_i * (mp_i - (m-8));  cnt = sum_i colcnt_i
        j3 = rowp.tile([1, N], F32, tag="j3")
        corr = scalp.tile([1, 1], F32, tag="corr")
        nc.vector.scalar_tensor_tensor(
            out=j3[:], in0=mp_fs[r][:], scalar=m8_s[r][:], in1=Cs[:],
            op0=ALU.subtract, op1=ALU.mult, accum_out=corr[:],
        )
        cnt = scalp.tile([1, 1], F32, tag="cnt")
        nc.vector.reduce_sum(cnt[:], Cs[:], axis=mybir.AxisListType.X)

        # S_all from the ACT accumulators
        sall_col = partp.tile([128, 1], F32, tag="sall_col")
        nc.vector.reduce_sum(sall_col[:], np_[:], axis=mybir.AxisListType.X)
        sall_ps = psC.tile([1, 1], F32, tag="tiny", name=f"sall_ps{r}")
        nc.tensor.matmul(sall_ps[:], sall_col[:], ones_col[:], start=True, stop=True)

        # S_diag = N*ln2 + sum_p - N*(m-8)
        sdiag = scalp.tile([1, 1], F32, tag="sdiag")
        nc.vector.scalar_tensor_tensor(
            out=sdiag[:], in0=m8_s[r][:], scalar=-float(N), in1=sum_p[r][:],
            op0=ALU.mult, op1=ALU.add,
        )
        nc.vector.tensor_scalar_add(sdiag[:], sdiag[:], float(N * math.log(2.0)))
        # num = (S_all - S_diag)/2 - corr
        haldif = scalp.tile([1, 1], F32, tag="haldif")
        nc.vector.tensor_sub(haldif[:], sall_ps[:], sdiag[:])
        num_fin = scalp.tile([1, 1], F32, tag="num_fin")
        nc.vector.scalar_tensor_tensor(
            out=num_fin[:], in0=haldif[:], scalar=0.5, in1=corr[:],
            op0=ALU.mult, op1=ALU.subtract,
        )

        # ll = (sum_p - slog)/N - m
        d1 = scalp.tile([1, 1], F32, tag="d1")
        nc.vector.tensor_sub(d1[:], sum_p[r][:], slog[:])
        ll = scalp.tile([1, 1], F32, tag="ll")
        nc.vector.scalar_tensor_tensor(
            out=ll[:], in0=d1[:], scalar=1.0 / N, in1=m_s[r][:],
            op0=ALU.mult, op1=ALU.subtract,
        )
        stt = scalp.tile([1, 3], F32, tag="stt")
        nc.scalar.copy(stt[0:1, 0:1], ll[:])
        nc.scalar.copy(stt[0:1, 1:2], num_fin[:])
        nc.scalar.copy(stt[0:1, 2:3], cnt[:])
        nc.sync.dma_start(out=st[r : r + 1, :], in_=stt[:])


_CACHED = None


def _build():
    global _CACHED
    if _CACHED is not None:
        return _CACHED
    nc = bacc.Bacc(
        "TRN2", target_bir_lowering=False, debug=False,
        enable_asserts=False, num_devices=NCORES,
    )
    yp = nc.dram_tensor("yp", [R, N], F32, kind="ExternalInput").ap()
    yt = nc.dram_tensor("yt", [R, N], F32, kind="ExternalInput").ap()
    mk = nc.dram_tensor("mk", [R, N], F32, kind="ExternalInput").ap()
    st = nc.dram_tensor("stats", [R, 3], F32, kind="ExternalOutput").ap()
    with tile.TileContext(nc) as tc, ExitStack() as ctx:
        _emit(tc, ctx, yp, yt, mk, st)
    nc.compile()
    _CACHED = nc
    return nc


def make_in_maps(y_pred, y_true, masks):
    return [
        {
            "yp": np.ascontiguousarray(y_pred[i * R : (i + 1) * R], dtype=np.float32),
            "yt": np.ascontiguousarray(y_true[i * R : (i + 1) * R], dtype=np.float32),
            "mk": np.ascontiguousarray(masks[i * R : (i + 1) * R], dtype=np.float32),
        }
        for i in range(NCORES)
    ]


def combine(stats, load_bal_loss):
    """stats: [NCORES, R, 3] -> scalar loss (matches reference combine)."""
    stats = np.asarray(stats, dtype=np.float64)
    ll = stats[:, :, 0].ravel()
    num = stats[:, :, 1].ravel()
    cnt = stats[:, :, 2].ravel()
    ranking = -np.mean(ll)
    pairwise = np.mean(num / (cnt + 1e-12))
    total = ranking + 0.3 * pairwise + 0.03 * float(np.squeeze(load_bal_loss))
    return np.float32(total)


def run_on_hw(in_maps, trace=False):
    nc = _build()
    return run_bass_kernel_spmd(nc, in_maps, list(range(NCORES)), trace=trace)


def kernel(y_pred, y_true, masks, load_bal_loss):
    res = run_on_hw(make_in_maps(y_pred, y_true, masks))
    stats = np.stack([res.results[i]["stats"] for i in range(NCORES)])
    return combine(stats, load_bal_loss)


# revision 15
# speedup vs baseline: 1.7105x; 1.0273x over previous
"""Trainium2 Bass kernel for DFFormerMoELoss (ListMLE + pairwise logistic + MoE).

Math (per batch row, N=2048):
  mp = y_pred * masks ; mt = y_true * masks
  ListMLE: with m = max(mp), E_k = exp(mp_k - m),
    S_i = sum_{k: mt_k <= mt_i} E_k   (sort-free suffix-logsumexp identity)
    ll = (sum(mp) - sum_i log S_i) / N - m
    ranking = -mean_rows(ll)
  Pairwise: pos[j,i] = (mt_i > mt_j). With E'_k = exp(mp_k - m + 8):
    softplus(mp_j - mp_i) = ln(E'_i + E'_j) + (m - 8 - mp_i)
    w_pre[j,i] = ln(E'_i + E'_j) is symmetric and pos[j,i] + pos[i,j] = 1
    for i != j (no ties), so
      sum pos*w_pre = (S_all - S_diag)/2,
      S_all = sum_{i,j} w_pre,  S_diag = N*ln2 + sum(mp) - N*(m-8)
    num = (S_all - S_diag)/2 - sum_i colcnt_i*(mp_i - m + 8),
      colcnt_i = sum_j pos[j,i]  (the reference's neg-branch term is
      annihilated by its final *pos factor)
    cnt = sum_i colcnt_i ; pairwise = mean_rows(num / (cnt + 1e-12))
  total = ranking + 0.3 * pairwise + 0.03 * load_bal_loss

Sharding: data-parallel, 2 rows per core across 8 cores; per-core output is
[2,3] = (ll, num, cnt) per row; host reduces the 16 row-scalars.
"""

import math
from contextlib import ExitStack

import numpy as np

import concourse.bacc as bacc
import concourse.bass as bass
import concourse.tile as tile
from concourse import mybir
from concourse.bass_utils import run_bass_kernel_spmd

B, N = 16, 2048
NCORES = 8
R = B // NCORES          # rows per core
NCH = N // 128           # 16 partition-chunks per row
NBLK = N // 512          # 4 psum column blocks
SHIFT = 8.0
F32 = mybir.dt.float32
BF16 = mybir.dt.bfloat16
AF = mybir.ActivationFunctionType
ALU = mybir.AluOpType


def _emit(tc: tile.TileContext, ctx: ExitStack, yp, yt, mk, st):
    nc = tc.nc
    consts = ctx.enter_context(tc.tile_pool(name="consts", bufs=1))
    rowp = ctx.enter_context(tc.tile_pool(name="rowp", bufs=1))
    row2p = ctx.enter_context(tc.tile_pool(name="row2p", bufs=2))
    repp = ctx.enter_context(tc.tile_pool(name="repp", bufs=2))
    partp = ctx.enter_context(tc.tile_pool(name="partp", bufs=2))
    workp = ctx.enter_context(tc.tile_pool(name="workp", bufs=3))
    scalp = ctx.enter_context(tc.tile_pool(name="scalp", bufs=4))
    psS = ctx.enter_context(tc.tile_pool(name="psS", bufs=6, space=bass.MemorySpace.PSUM))
    psC = ctx.enter_context(tc.tile_pool(name="psC", bufs=2, space=bass.MemorySpace.PSUM))

    ones_col = consts.tile([128, 1], F32, tag="ones_col")
    nc.vector.memset(ones_col[:], 1.0)

    # ---------------- Phase A: load, mask, broadcast, exp (both rows) -------
    T_rep, E8_rep, T_part, P_part, E8_part = [], [], [], [], []
    EO, E_f, m_s, m8_s, sum_p, mp_fs = [], [], [], [], [], []
    for r in range(R):
        pf = row2p.tile([1, N], F32, tag="raw")
        tf = row2p.tile([1, N], F32, tag="raw", name="tf")
        mf = rowp.tile([1, N], F32, tag="mf")
        nc.sync.dma_start(out=pf[:], in_=yp[r : r + 1, :])
        nc.sync.dma_start(out=tf[:], in_=yt[r : r + 1, :])
        nc.sync.dma_start(out=mf[:], in_=mk[r : r + 1, :])
        mp_f = row2p.tile([1, N], F32, tag="mp_f")
        mt_f = rowp.tile([1, N], F32, tag="mt_f")
        nc.vector.tensor_mul(mp_f[:], pf[:], mf[:])
        nc.vector.tensor_mul(mt_f[:], tf[:], mf[:])

        # partition-major layouts: element (c*128+q) -> [q, c]
        pq = partp.tile([128, NCH], F32, tag="pq")
        tq = partp.tile([128, NCH], F32, tag="tq")
        mq = partp.tile([128, NCH], F32, tag="mq")
        nc.sync.dma_start(out=pq[:], in_=yp[r].rearrange("(c p) -> p c", p=128))
        nc.sync.dma_start(out=tq[:], in_=yt[r].rearrange("(c p) -> p c", p=128))
        nc.sync.dma_start(out=mq[:], in_=mk[r].rearrange("(c p) -> p c", p=128))
        Pp = partp.tile([128, NCH], F32, tag="P_part")
        Tp = partp.tile([128, NCH], F32, tag="T_part")
        nc.vector.tensor_mul(Pp[:], pq[:], mq[:])
        nc.vector.tensor_mul(Tp[:], tq[:], mq[:])

        # row max m and derived scalars
        m1 = scalp.tile([1, 1], F32, tag="m1")
        nc.vector.reduce_max(m1[:], mp_f[:], axis=mybir.AxisListType.X)
        m8 = scalp.tile([1, 1], F32, tag="m8")
        nc.vector.tensor_scalar_add(m8[:], m1[:], -SHIFT)       # m - 8
        mm2 = scalp.tile([1, 2], F32, tag="mm2")
        nc.vector.tensor_scalar_mul(mm2[0:1, 0:1], m1[:], -1.0)  # -m
        nc.vector.tensor_scalar_mul(mm2[0:1, 1:2], m8[:], -1.0)  # 8 - m
        mcols = partp.tile([128, 2], F32, tag="mcols")
        nc.gpsimd.partition_broadcast(mcols[:], mm2[:])
        mneg, mneg8 = mcols[:, 0:1], mcols[:, 1:2]

        # sum(mp) via partition layout
        spc = partp.tile([128, 1], F32, tag="spc")
        nc.vector.reduce_sum(spc[:], Pp[:], axis=mybir.AxisListType.X)
        spp = psC.tile([1, 1], F32, tag="tiny", name=f"spp{r}")
        nc.tensor.matmul(spp[:], spc[:], ones_col[:], start=True, stop=True)
        sp = scalp.tile([1, 1], F32, tag="sum_p")
        nc.scalar.copy(sp[:], spp[:])

        # exps: unshifted for ListMLE, shifted (+8) for pairwise ln
        Epf = partp.tile([128, NCH], F32, tag="E_partf")
        nc.scalar.activation(Epf[:], Pp[:], AF.Exp, bias=mneg, scale=1.0)
        E8p = partp.tile([128, NCH], F32, tag="E8_part")
        nc.scalar.activation(E8p[:], Pp[:], AF.Exp, bias=mneg8, scale=1.0)
        Ef = row2p.tile([1, N], F32, tag="E_f")
        nc.scalar.activation(Ef[:], mp_f[:], AF.Exp, bias=mm2[0:1, 0:1], scale=1.0)
        E8f = rowp.tile([1, N], F32, tag="E8_f")
        nc.scalar.activation(E8f[:], mp_f[:], AF.Exp, bias=mm2[0:1, 1:2], scale=1.0)
        # interleaved bf16 lhsT: col 2c = E, col 2c+1 = 1  (contiguous slices)
        Eo = partp.tile([128, 2 * NCH], BF16, tag="EO")
        nc.vector.tensor_copy(Eo[:, 0 : 2 * NCH : 2], Epf[:])
        nc.vector.memset(Eo[:, 1 : 2 * NCH : 2], 1.0)

        # replicate rows across 128 partitions on the (idle) Pool engine
        Tr = repp.tile([128, N], F32, tag="T_rep")
        nc.gpsimd.partition_broadcast(Tr[:], mt_f[:])
        E8r = repp.tile([128, N], F32, tag="E8_rep")
        nc.gpsimd.partition_broadcast(E8r[:], E8f[:])

        T_rep.append(Tr); E8_rep.append(E8r); T_part.append(Tp); P_part.append(Pp)
        E8_part.append(E8p); EO.append(Eo); E_f.append(Ef)
        m_s.append(m1); m8_s.append(m8); sum_p.append(sp); mp_fs.append(mp_f)

    # ------- Phase B (NxN sweep) + Phase D (stats), interleaved per row -----
    for r in range(R):
        np_ = partp.tile([128, NCH], F32, tag="num_part")
        dp_ = partp.tile([128, NCH], F32, tag="diag_part")
        sc_ps = [
            psS.tile([2, 512], F32, tag="sc", name=f"sc_r{r}b{b}")
            for b in range(NBLK)
        ]
        for c in range(NCH):
            i0 = 128 * c
            pos = workp.tile([128, N], BF16, tag="pos")
            nc.vector.tensor_scalar(
                out=pos[:], in0=T_rep[r][:], scalar1=T_part[r][:, c : c + 1],
                scalar2=None, op0=ALU.is_gt,
            )
            w = workp.tile([128, N], BF16, tag="w")
            nc.scalar.activation(
                w[:, 0 : N - i0], E8_rep[r][:, i0:N], AF.Ln,
                bias=E8_part[r][:, c : c + 1],
                scale=1.0, accum_out=np_[:, c : c + 1],
            )
            wd = workp.tile([128, 128], BF16, tag="wd")
            nc.scalar.activation(
                wd[:], E8_rep[r][:, i0 : i0 + 128], AF.Ln,
                bias=E8_part[r][:, c : c + 1],
                scale=1.0, accum_out=dp_[:, c : c + 1],
            )
            for b in range(NBLK):
                nc.tensor.matmul(
                    sc_ps[b][:], EO[r][:, 2 * c : 2 * c + 2],
                    pos[:, b * 512 : (b + 1) * 512],
                    start=(c == 0), stop=(c == NCH - 1),
                )

        # drain psum: row0 = SumE (add diagonal E_i), row1 = colcnt
        Ss = rowp.tile([1, N], F32, tag="SumE_sb")
        Cs = rowp.tile([1, N], F32, tag="colcnt_sb")
        for b in range(NBLK):
            sl = slice(b * 512, (b + 1) * 512)
            nc.vector.tensor_add(Ss[0:1, sl], sc_ps[b][0:1, :], E_f[r][0:1, sl])
            C2 = rowp.tile([2, 512], F32, tag="C2", name=f"C2_{r}{b}")
            nc.vector.tensor_copy(C2[:, :], sc_ps[b][:, :])
            nc.sync.dma_start(out=Cs[0:1, sl], in_=C2[1:2, :])

        # ---- stats ----
        # slog = sum_i ln(S_i)  (Ln in place, accum gives the sum)
        slog = scalp.tile([1, 1], F32, tag="slog")
        nc.scalar.activation(Ss[:], Ss[:], AF.Ln, accum_out=slog[:])

        # corr = sum_i colcnt_i * (mp_i - (m-8));  cnt = sum_i colcnt_i
        j3 = rowp.tile([1, N], F32, tag="j3")
        corr = scalp.tile([1, 1], F32, tag="corr")
        nc.vector.scalar_tensor_tensor(
            out=j3[:], in0=mp_fs[r][:], scalar=m8_s[r][:], in1=Cs[:],
            op0=ALU.subtract, op1=ALU.mult, accum_out=corr[:],
        )
        cnt = scalp.tile([1, 1], F32, tag="cnt")
        nc.vector.reduce_sum(cnt[:], Cs[:], axis=mybir.AxisListType.X)

        # S_comp and D_full from the ACT accumulators
        sall_col = partp.tile([128, 1], F32, tag="sall_col")
        nc.vector.reduce_sum(sall_col[:], np_[:], axis=mybir.AxisListType.X)
        sall_ps = psC.tile([1, 1], F32, tag="tiny", name=f"sall_ps{r}")
        nc.tensor.matmul(sall_ps[:], sall_col[:], ones_col[:], start=True, stop=True)
        dall_col = partp.tile([128, 1], F32, tag="dall_col")
        nc.vector.reduce_sum(dall_col[:], dp_[:], axis=mybir.AxisListType.X)
        dall_ps = psC.tile([1, 1], F32, tag="tiny", name=f"dall_ps{r}")
        nc.tensor.matmul(dall_ps[:], dall_col[:], ones_col[:], start=True, stop=True)

        # S_diag = N*ln2 + sum_p - N*(m-8)
        sdiag = scalp.tile([1, 1], F32, tag="sdiag")
        nc.vector.scalar_tensor_tensor(
            out=sdiag[:], in0=m8_s[r][:], scalar=-float(N), in1=sum_p[r][:],
            op0=ALU.mult, op1=ALU.add,
        )
        nc.vector.tensor_scalar_add(sdiag[:], sdiag[:], float(N * math.log(2.0)))
        # num = S_comp - (D_full + S_diag)/2 - corr
        dd = scalp.tile([1, 1], F32, tag="dd")
        nc.vector.tensor_add(dd[:], dall_ps[:], sdiag[:])
        haldif = scalp.tile([1, 1], F32, tag="haldif")
        nc.vector.scalar_tensor_tensor(
            out=haldif[:], in0=dd[:], scalar=-0.5, in1=sall_ps[:],
            op0=ALU.mult, op1=ALU.add,
        )
        num_fin = scalp.tile([1, 1], F32, tag="num_fin")
        nc.vector.tensor_sub(num_fin[:], haldif[:], corr[:])

        # ll = (sum_p - slog)/N - m
        d1 = scalp.tile([1, 1], F32, tag="d1")
        nc.vector.tensor_sub(d1[:], sum_p[r][:], slog[:])
        ll = scalp.tile([1, 1], F32, tag="ll")
        nc.vector.scalar_tensor_tensor(
            out=ll[:], in0=d1[:], scalar=1.0 / N, in1=m_s[r][:],
            op0=ALU.mult, op1=ALU.subtract,
        )
        stt = scalp.tile([1, 3], F32, tag="stt")
        nc.scalar.copy(stt[0:1, 0:1], ll[:])
        nc.scalar.copy(stt[0:1, 1:2], num_fin[:])
        nc.scalar.copy(stt[0:1, 2:3], cnt[:])
        nc.sync.dma_start(out=st[r : r + 1, :], in_=stt[:])


_CACHED = None


def _build():
    global _CACHED
    if _CACHED is not None:
        return _CACHED
    nc = bacc.Bacc(
        "TRN2", target_bir_lowering=False, debug=False,
        enable_asserts=False, num_devices=NCORES,
    )
    yp = nc.dram_tensor("yp", [R, N], F32, kind="ExternalInput").ap()
    yt = nc.dram_tensor("yt", [R, N], F32, kind="ExternalInput").ap()
    mk = nc.dram_tensor("mk", [R, N], F32, kind="ExternalInput").ap()
    st = nc.dram_tensor("stats", [R, 3], F32, kind="ExternalOutput").ap()
    with tile.TileContext(nc) as tc, ExitStack() as ctx:
        _emit(tc, ctx, yp, yt, mk, st)
    nc.compile()
    _CACHED = nc
    return nc


def make_in_maps(y_pred, y_true, masks):
    return [
        {
            "yp": np.ascontiguousarray(y_pred[i * R : (i + 1) * R], dtype=np.float32),
            "yt": np.ascontiguousarray(y_true[i * R : (i + 1) * R], dtype=np.float32),
            "mk": np.ascontiguousarray(masks[i * R : (i + 1) * R], dtype=np.float32),
        }
        for i in range(NCORES)
    ]


def combine(stats, load_bal_loss):
    """stats: [NCORES, R, 3] -> scalar loss (matches reference combine)."""
    stats = np.asarray(stats, dtype=np.float64)
    ll = stats[:, :, 0].ravel()
    num = stats[:, :, 1].ravel()
    cnt = stats[:, :, 2].ravel()
    ranking = -np.mean(ll)
    pairwise = np.mean(num / (cnt + 1e-12))
    total = ranking + 0.3 * pairwise + 0.03 * float(np.squeeze(load_bal_loss))
    return np.float32(total)


def run_on_hw(in_maps, trace=False):
    nc = _build()
    return run_bass_kernel_spmd(nc, in_maps, list(range(NCORES)), trace=trace)


def kernel(y_pred, y_true, masks, load_bal_loss):
    res = run_on_hw(make_in_maps(y_pred, y_true, masks))
    stats = np.stack([res.results[i]["stats"] for i in range(NCORES)])
    return combine(stats, load_bal_loss)
